# revision 31
# baseline (speedup 1.0000x reference)
"""BerHu loss kernel for Trainium2, 8-core data-parallel.

Reference computation (per sample n over its S = 1*480*640 elements):
    d  = pred - tgt
    c  = max|d| / 5
    berhu = |d|                 where |d| <= c
          = (d^2 + c^2) / (2c)  otherwise
    loss = mean_n mean_i berhu

Identity used on device:  berhu = |d| + relu(|d| - c)^2 * (1/(2c))
Two custom DVE ops do all heavy lifting (one pass each over the data):
  ABSDIFF:  ad = |p - t|            accum: mx = max(ad)      [per partition]
  BERHU:    junk = ad + relu(ad-c)^2 * i2c   accum: bh = sum [per partition]
The host sums the per-partition/per-sample bh partials:
    loss = sum(bh) / (N * S).

Sharding: pure data parallel, 8 samples per core on 8 cores; each
sample's 307200 elements are laid out [128 partitions x 2400].
"""

import numpy as np

N = 64          # batch
S = 307200      # 1*480*640 elements per sample
NCORES = 8
NLOC = N // NCORES   # samples per core
P = 128              # SBUF partitions
F = S // P           # 2400 columns per sample

_PROG = None


def _register_ops():
    import concourse.dve_ops as dve_ops
    from concourse.dve_ops import OPS, DveOp, has_src1
    from concourse.dve_spec import (C0, C1, C2, AluOp, Spec, Src0, Src1,
                                    Zero, lower)
    from concourse.dve_spec import relu, sq, maxx
    from concourse.dve_uop import DveOpSpec

    def add_op(name, spec):
        for o in OPS:
            if o.name == name:
                return o
        op = DveOp(name, spec, subdim=False, uops_sha={})
        OPS.append(op)
        dve_ops.CUSTOM_DVE_SPECS[name] = spec
        dve_ops._SUB_OPCODE_FOR_NAME[name] = (
            dve_ops._CUSTOM_DVE_ROW_BASE + len(OPS) - 1)
        assert dve_ops._SUB_OPCODE_FOR_NAME[name] < 0x20
        for ver in ("v3", "v4"):
            sha = DveOpSpec(
                name=name,
                opcode=dve_ops.get_dve_sub_opcode(name),
                uops=lower(spec, ver=ver),
                rd1_en=has_src1(spec),
            ).sha(ver)
            op.uops_sha[ver] = sha
        return op

    def _absdiff_ref(in0, in1, c0, c1, c2):
        x = in0.astype(np.float32).reshape(in0.shape[0], -1)
        y = np.asarray(in1, np.float32).reshape(in0.shape[0], -1)
        out = np.abs(x - y).astype(np.float32)
        return out, out.max(axis=-1)

    def _berhu_ref(in0, in1, c0, c1, c2):
        # c0 = c = m/5, c1 = 1/(2c) = 2.5/m
        x = in0.astype(np.float32).reshape(in0.shape[0], -1)
        r = np.maximum(x - c0, 0.0).astype(np.float32)
        out = (x + r * r * c1).astype(np.float32)
        return out, out.sum(axis=-1, dtype=np.float32)

    d = Src0 - Src1
    absdiff = add_op(
        "ANT_BERHU_ABSDIFF",
        Spec(body=maxx(d, Zero - d), accum=AluOp.MAX, reference=_absdiff_ref),
    )
    berhu = add_op(
        "ANT_BERHU_ACC",
        Spec(body=Src0 + sq(relu(Src0 - C0)) * C1, accum=AluOp.ADD,
             reference=_berhu_ref),
    )
    return absdiff, berhu


def _build(repeat=1, loop_n=None, queues=("sync", "scalar"),
           tail_opt=False, io_bufs=3, junk_bufs=2, single_loads=False,
           chain_batch=False, ad_bf16=False):
    """Build the per-core program. `repeat` > 1 replays the whole 8-sample
    body that many times inside one NEFF (unrolled); `loop_n` instead wraps
    the body in a device-side For_i loop (benchmarking only). `queues`:
    engine queues the paired input DMAs rotate over. `tail_opt`: load the
    last two samples as singles ([2,2,2,1,1] grouping) so only one
    sample's DVE work remains after the final DMA byte lands, shortening
    the single-shot pipeline drain by ~3 us."""
    from contextlib import ExitStack

    import concourse.bacc as bacc
    import concourse.tile as tile
    from concourse import mybir

    absdiff_op, berhu_op = _register_ops()

    f32 = mybir.dt.float32
    adt = mybir.dt.bfloat16 if ad_bf16 else f32
    Alu = mybir.AluOpType

    nc = bacc.Bacc("TRN2", target_bir_lowering=False, debug=False,
                   num_devices=NCORES)
    p_d = nc.dram_tensor("p", [NLOC * P, F], f32, kind="ExternalInput").ap()
    t_d = nc.dram_tensor("t", [NLOC * P, F], f32, kind="ExternalInput").ap()
    bh_d = nc.dram_tensor("bh", [P, NLOC], f32, kind="ExternalOutput").ap()

    with tile.TileContext(nc) as tc, ExitStack() as ctx:
        if chain_batch:
            # bf16 ad/junk halves the DVE<->SBUF bank traffic and frees
            # enough SBUF for a 4-deep DMA runway
            io_bufs, junk_bufs, work_bufs, psum_bufs = (
                (4, 1, 6, 2) if ad_bf16 else (3, 1, 6, 2))
        else:
            work_bufs, psum_bufs = 3, 3
        io = ctx.enter_context(tc.tile_pool(name="io", bufs=io_bufs))
        work = ctx.enter_context(tc.tile_pool(name="work", bufs=work_bufs))
        work2 = ctx.enter_context(tc.tile_pool(name="work2", bufs=junk_bufs))
        small = ctx.enter_context(tc.tile_pool(name="small", bufs=3))
        stats = ctx.enter_context(tc.tile_pool(name="stats", bufs=1))
        psum = ctx.enter_context(tc.tile_pool(name="psum", bufs=psum_bufs,
                                              space="PSUM"))

        bh_t = stats.tile([P, NLOC], f32, tag="bh")
        ones_t = stats.tile([1, P], f32, tag="ones")
        nc.vector.memset(ones_t[:], 1.0)
        # identity matrix for PE cross-partition transpose
        ident = stats.tile([P, P], f32, tag="ident")
        nc.vector.memset(ident[:], 1.0)
        nc.gpsimd.affine_select(
            out=ident[:], in_=ident[:], pattern=[[-1, P]],
            compare_op=mybir.AluOpType.is_equal, fill=0.0,
            base=0, channel_multiplier=1,
        )
        total = NLOC * repeat
        q_engines = [getattr(nc, q) for q in queues]

        pair = {}

        def load_pair(i):
            # one 2.4 MB DMA covers two consecutive samples (halves the
            # per-sample DMA instruction + completion-sem overhead)
            n = i % NLOC
            rows = slice(n * P, (n + 2) * P)
            pt = io.tile([P, 2 * F], f32, tag="p")
            tt = io.tile([P, 2 * F], f32, tag="t")
            src_p = p_d[rows, :].rearrange("(s p) f -> p s f", s=2)
            src_t = t_d[rows, :].rearrange("(s p) f -> p s f", s=2)
            q_engines[i % len(q_engines)].dma_start(
                out=pt[:].rearrange("p (s f) -> p s f", s=2), in_=src_p)
            q_engines[(i + 1) % len(q_engines)].dma_start(
                out=tt[:].rearrange("p (s f) -> p s f", s=2), in_=src_t)
            pair["p"], pair["t"] = pt, tt

        singles = {}

        def load_single(n):
            # dedicated [P, F] tiles for a tail sample (1.2 MB DMAs)
            rows = slice((n % NLOC) * P, (n % NLOC + 1) * P)
            pt = stats.tile([P, F], f32, tag=f"s{n % NLOC}p")
            tt = stats.tile([P, F], f32, tag=f"s{n % NLOC}t")
            q_engines[0].dma_start(out=pt[:], in_=p_d[rows, :])
            q_engines[1 % len(q_engines)].dma_start(
                out=tt[:], in_=t_d[rows, :])
            singles[n % NLOC] = {"p": pt, "t": tt}

        def load_one(i):
            # per-sample 1.2 MB DMAs (finer issue granularity than pairs)
            n = i % NLOC
            rows = slice(n * P, (n + 1) * P)
            pt = io.tile([P, F], f32, tag="p")
            tt = io.tile([P, F], f32, tag="t")
            q_engines[i % len(q_engines)].dma_start(out=pt[:],
                                                    in_=p_d[rows, :])
            q_engines[(i + 1) % len(q_engines)].dma_start(out=tt[:],
                                                          in_=t_d[rows, :])
            pair["p"], pair["t"] = pt, tt

        def pass1(i, single=False):
            if single:
                src_p = singles[i % NLOC]["p"][:, :]
                src_t = singles[i % NLOC]["t"][:, :]
            elif single_loads:
                load_one(i)
                src_p = pair["p"][:, :]
                src_t = pair["t"][:, :]
            else:
                if i % 2 == 0:
                    load_pair(i)
                k = i % 2
                cols = slice(k * F, (k + 1) * F)
                src_p = pair["p"][:, cols]
                src_t = pair["t"][:, cols]
            # ad = |p - t|; mxn = per-partition max(ad)
            ad = work.tile([P, F], adt, tag="ad")
            mxn = small.tile([P, 1], f32, tag="mxn")
            nc.vector._custom_dve(absdiff_op, out=ad[:],
                                  in0=src_p, in1=src_t,
                                  accum_out=mxn[:])
            return {"ad": ad, "mxn": mxn}

        def chain(st):
            # cross-partition max: PE transpose (mxn^T @ I) -> [1, P] PSUM
            # row, DVE max-reduce -> scalar m; cpair = (m/5, 2.5/m); a K=1
            # ones-matmul broadcasts cpair to all 128 partitions in PSUM.
            mrow = psum.tile([1, P], f32, tag="mrow")
            nc.tensor.matmul(mrow[:], st["mxn"][:], ident[:],
                             start=True, stop=True)
            cpair = small.tile([1, 2], f32, tag="cpair")
            mr = small.tile([1, 3], f32, tag="mr")
            nc.vector.tensor_reduce(out=mr[0:1, 0:1], in_=mrow[:],
                                    axis=mybir.AxisListType.X, op=Alu.max)
            # floor m to avoid 1/0 when pred == tgt exactly (then bh = 0
            # correctly, since relu(0 - c) = 0)
            nc.vector.tensor_scalar_max(out=mr[0:1, 1:2],
                                        in0=mr[0:1, 0:1], scalar1=1e-20)
            nc.vector.reciprocal(out=mr[0:1, 2:3], in_=mr[0:1, 1:2])
            nc.vector.tensor_scalar_mul(out=cpair[0:1, 0:1],
                                        in0=mr[0:1, 1:2], scalar1=0.2)
            nc.vector.tensor_scalar_mul(out=cpair[0:1, 1:2],
                                        in0=mr[0:1, 2:3], scalar1=2.5)
            cb = psum.tile([P, 2], f32, tag="cb")
            nc.tensor.matmul(cb[:], ones_t[:], cpair[:],
                             start=True, stop=True)
            st["cb"] = cb

        def pass2(i, st):
            # bh[:, n] = sum(ad + relu(ad - c)^2 * i2c)
            n = i % NLOC
            junk = work2.tile([P, F], adt, tag="junk")
            nc.vector._custom_dve(berhu_op, out=junk[:], in0=st["ad"][:],
                                  s0=st["cb"][:, 0:1], s1=st["cb"][:, 1:2],
                                  accum_out=bh_t[:, n:n + 1])

        # --- batched-chain variant: one c-derivation per 4 samples.
        # Cuts the per-sample chain's ~5 small DVE ops (+ their PE/DVE
        # semaphore round trips) off the DVE queue, which the NTFF trace
        # shows is the marginal bottleneck (DVE busy ~46.6us/body vs the
        # ~45.2us fabric-rate DMA floor).
        if chain_batch:
            mxa = stats.tile([P, 4], f32, tag="mxa")
            mxb = stats.tile([P, 4], f32, tag="mxb")

        def pass1b(i):
            if i % 2 == 0:
                load_pair(i)
            k = i % 2
            cols = slice(k * F, (k + 1) * F)
            ad = work.tile([P, F], adt, tag="ad")
            n = i % NLOC
            mx = mxa if n < 4 else mxb
            nc.vector._custom_dve(absdiff_op, out=ad[:],
                                  in0=pair["p"][:, cols],
                                  in1=pair["t"][:, cols],
                                  accum_out=mx[:, n % 4:n % 4 + 1])
            return ad

        def chain4_front(mx):
            # cross-partition fold of the 4 per-sample maxes into a [1,4]
            # PSUM row. PE matmuls need base partition 0, so everything
            # goes through free-dim rows rather than per-partition slices.
            mrow4 = psum.tile([4, P], f32, tag="mrow4")
            nc.tensor.matmul(mrow4[:], mx[:], ident[:], start=True,
                             stop=True)
            mx4 = small.tile([4, 1], f32, tag="mx4")
            nc.vector.tensor_reduce(out=mx4[:], in_=mrow4[:],
                                    axis=mybir.AxisListType.X, op=Alu.max)
            mxT = psum.tile([1, 4], f32, tag="mxT")
            nc.tensor.matmul(mxT[:], mx4[:], ident[0:4, 0:4], start=True,
                             stop=True)
            return mxT

        def chain4_back(mxT):
            # c-derivation for 4 samples from the folded max row
            row = small.tile([1, 12], f32, tag="row")
            ci = small.tile([1, 8], f32, tag="ci")
            nc.vector.tensor_copy(row[:, 0:4], mxT[:])
            nc.vector.tensor_scalar_max(out=row[:, 4:8], in0=row[:, 0:4],
                                        scalar1=1e-20)
            nc.vector.reciprocal(out=row[:, 8:12], in_=row[:, 4:8])
            nc.vector.tensor_scalar_mul(out=ci[:, 0:4], in0=row[:, 4:8],
                                        scalar1=0.2)
            nc.vector.tensor_scalar_mul(out=ci[:, 4:8], in0=row[:, 8:12],
                                        scalar1=2.5)
            # one K=1 matmul broadcasts all 4 samples' (c, i2c) to [P, 8]
            cb_all = psum.tile([P, 8], f32, tag="cb")
            nc.tensor.matmul(cb_all[:], ones_t[:], ci[:], start=True,
                             stop=True)
            return cb_all

        def chain4(mx):
            return chain4_back(chain4_front(mx))

        def pass2b(i, ad, cb_all):
            n = i % NLOC
            junk = work2.tile([P, F], adt, tag="junk")
            nc.vector._custom_dve(berhu_op, out=junk[:], in0=ad[:],
                                  s0=cb_all[:, n % 4:n % 4 + 1],
                                  s1=cb_all[:, 4 + n % 4:5 + n % 4],
                                  accum_out=bh_t[:, n:n + 1])

        def body_batched():
            # p1(0..3) chainA [p2(0) p1(4)] .. [p2(3) p1(7)] chainB p2(4..7)
            # pass1s stay interleaved through the pass2 stretch so the DMA
            # stream keeps being consumed (io pool never stalls the queues).
            # chain_batch == 2 additionally pulls p1(4) ahead of chainA.
            # chain_batch == 3 splits each chain around its PE transpose
            # and slots a pass between the halves: the NTFF trace shows
            # ~4us DVE stalls right after each chain's tensor_reduce,
            # waiting on the PE round trip with nothing else queued.
            if chain_batch == 3:
                for b in range(0, total, NLOC):
                    ads = {}
                    for j in range(4):
                        ads[j] = pass1b(b + j)
                    mxT_a = chain4_front(mxa)
                    ads[4] = pass1b(b + 4)      # hides A's PE round trip
                    cb_a = chain4_back(mxT_a)
                    for j in range(3):
                        pass2b(b + j, ads.pop(j), cb_a)
                        ads[5 + j] = pass1b(b + 5 + j)
                    mxT_b = chain4_front(mxb)
                    pass2b(b + 3, ads.pop(3), cb_a)  # hides B's round trip
                    cb_b = chain4_back(mxT_b)
                    for j in range(4):
                        pass2b(b + 4 + j, ads.pop(4 + j), cb_b)
                return
            early = 1 if chain_batch == 2 else 0
            assert total % NLOC == 0
            for b in range(0, total, NLOC):
                ads = {}
                for j in range(4 + early):
                    ads[j] = pass1b(b + j)
                cb_a = chain4(mxa)
                for j in range(4):
                    pass2b(b + j, ads.pop(j), cb_a)
                    nxt = 4 + early + j
                    if nxt < NLOC:
                        ads[nxt] = pass1b(b + nxt)
                cb_b = chain4(mxb)
                for j in range(4):
                    pass2b(b + 4 + j, ads.pop(4 + j), cb_b)

        # 2-deep software pipeline: pass1(i) | chain(i-1) | pass2(i-2) keeps
        # the DVE stream free of waits on the c-derivation chain.
        def body():
            if chain_batch:
                return body_batched()
            if not tail_opt:
                hist = {}
                for i in range(total):
                    hist[i] = pass1(i)
                    if i - 1 >= 0:
                        chain(hist[i - 1])
                    if i - 2 >= 0:
                        pass2(i - 2, hist.pop(i - 2))
                for i in (total - 2, total - 1):
                    if i >= 0:
                        if "cb" not in hist[i]:
                            chain(hist[i])
                        pass2(i, hist.pop(i))
                return
            # tail-lean: samples 0-5 as pairs, 6 and 7 as singles whose
            # DMAs trail the stream; the DVE finishes samples 0-6 before
            # the last byte lands, leaving only sample 7's work as drain.
            assert total % NLOC == 0
            for b in range(0, total, NLOC):
                hist = {}
                for j in range(6):
                    hist[j] = pass1(b + j)
                    if j == 4:
                        load_single(b + 6)
                        load_single(b + 7)
                    if j - 1 >= 0:
                        chain(hist[j - 1])
                    if j - 2 >= 0:
                        pass2(b + j - 2, hist.pop(j - 2))
                chain(hist[5])
                pass2(b + 4, hist.pop(4))
                pass2(b + 5, hist.pop(5))
                for j in (6, 7):
                    st = pass1(b + j, single=True)
                    chain(st)
                    pass2(b + j, st)

        if loop_n is not None:
            with tc.For_i(0, loop_n, 1):
                body()
        else:
            body()

        nc.sync.dma_start(out=bh_d[:], in_=bh_t[:])

    nc.compile()
    return nc


def _get_prog():
    global _PROG
    if _PROG is None:
        # chain_batch: one c-derivation chain per 4 samples instead of per
        # sample — cuts ~5.5us/body of small ops + semaphore round trips
        # off the DVE queue, which the NTFF trace showed sat marginally
        # above the DMA fabric floor (46.6us vs 45.2us per body). Measured
        # 3.5us/body faster than the per-sample chain (interleaved A/B).
        # ad_bf16: intermediate |p-t| and the dead elementwise pass-2
        # output stored as bf16 — same DVE cycles (1x mode), half the
        # DVE<->SBUF bank traffic alongside the SDMA write stream, and the
        # freed SBUF deepens the DMA runway to io_bufs=4. Measured another
        # 6.1us/body (loss rel err ~3e-4 from ad quantization, vs the
        # 2e-2 gate).
        _PROG = _build(chain_batch=True, ad_bf16=True)
    return _PROG


def _combine(results):
    total = 0.0
    for r in results:
        total += r["bh"].astype(np.float64).sum()
    return np.float32(total / (N * S))


def kernel(predictions, targets):
    from concourse.bass_utils import run_bass_kernel_spmd

    nc = _get_prog()
    p = np.ascontiguousarray(
        np.asarray(predictions, dtype=np.float32).reshape(NCORES, NLOC * P, F))
    t = np.ascontiguousarray(
        np.asarray(targets, dtype=np.float32).reshape(NCORES, NLOC * P, F))
    in_maps = [{"p": p[k], "t": t[k]} for k in range(NCORES)]
    res = run_bass_kernel_spmd(nc, in_maps, list(range(NCORES)))
    return _combine(res.results)

